# revision 10
# baseline (speedup 1.0000x reference)
"""CAM (channel attention) kernel for Trainium2, SPMD over 8 NeuronCores.

Problem: x [16, 512, 64, 64] fp32, gamma [1] fp32.
  q = x.reshape(B, C, N);  energy = q @ q^T          (C x C, contract over N=4096)
  attention = softmax(max(energy, -1, keepdims) - energy, -1)
  out = attention @ q;  result = gamma * out + x

Sharding: data-parallel over batch. 16 batches / 8 cores = 2 batches per core.
gamma replicated. Each core computes its own C x C attention per batch.

Math: energy is symmetric, and
  softmax(m[c] - energy[c, :]) = exp(mn[c] - energy[c, :]) / sum(...)
with mn[c] = min_d energy[c, d] (jax softmax's internal max-shift turns the
row-max of (m - e) into the row-min of e). All exp args <= 0 -> no overflow.

All transposes run on the TensorEngine in transpose-mode (DMA xbar-transposes
measured ~8us each here and throttle the global in-flight DMA window, which
starves the x loads). mm1 is software-pipelined per 128-wide n-chunk:
  PE:  [transpose chunk k (4x 128x128)] [matmuls chunk k-1 (4x N=512)] ...
with tiny DVE copies draining each transposed chunk from PSUM to SBUF.

Engine layout:
  SP ring : x loads (plain, back-to-back)
  ACT ring: out stores; ACT compute: exp with fused row-sum
  DVE     : fp32->bf16 casts, qT/AT psum->sbuf copies, row-min,
            epilogue (psum*rg)+x as one fused scalar_tensor_tensor
  PE      : qT transposes, mm1, AT transposes, mm2
"""

import sys

if "/opt/trn_rl_repo" not in sys.path:
    sys.path.insert(0, "/opt/trn_rl_repo")

import numpy as np

import concourse.bacc as bacc
import concourse.mybir as mybir
import concourse.tile as tile
from concourse.bass_utils import run_bass_kernel_spmd
from concourse.masks import make_identity

# Problem constants (hardcoded; kernel.py must be self-contained).
B, C, H, W = 16, 512, 64, 64
N = H * W                      # 4096
N_CORES = 8
BPC = B // N_CORES             # batches per core = 2
CB = C // 128                  # c-blocks = 4
NK = N // 128                  # contraction chunks for mm1 = 32
NT = N // 512                  # mm2 output tiles per c-block = 8

F32 = mybir.dt.float32
BF16 = mybir.dt.bfloat16

_PROGRAM = None


def _build_program():
    nc = bacc.Bacc("TRN2", target_bir_lowering=False, debug=True)
    x = nc.declare_dram_parameter("x", [BPC, C, N], F32, isOutput=False)
    gamma = nc.declare_dram_parameter("gamma", [1], F32, isOutput=False)
    out = nc.declare_dram_parameter("out", [BPC, C, N], F32, isOutput=True)

    with tile.TileContext(nc) as tc:
        with (
            tc.tile_pool(name="xf", bufs=20) as xf_pool,
            tc.tile_pool(name="qbf", bufs=12) as qbf_pool,
            tc.tile_pool(name="qts", bufs=3) as qts_pool,
            tc.tile_pool(name="attn", bufs=4) as a_pool,
            tc.tile_pool(name="att", bufs=20) as at_pool,
            tc.tile_pool(name="stat", bufs=24) as stat_pool,
            tc.tile_pool(name="stage", bufs=6) as stage_pool,
            tc.tile_pool(name="const", bufs=1) as const_pool,
            tc.tile_pool(name="ps1", bufs=4, space="PSUM") as ps1_pool,
            tc.tile_pool(name="ps2", bufs=4, space="PSUM") as ps2_pool,
        ):
            # constants: gamma broadcast + identity for PE transposes
            gsb = const_pool.tile([1, 1], F32, tag="gsb", name="gsb")
            nc.sync.dma_start(gsb[:, :], gamma[None, :])
            gb = const_pool.tile([128, 1], F32, tag="gb", name="gb")
            nc.gpsimd.partition_broadcast(gb[:, :], gsb[:, :])
            ident = const_pool.tile([128, 128], BF16, tag="ident", name="ident")
            make_identity(nc, ident[:, :])

            # per-batch state
            x_tiles = [{} for _ in range(BPC)]
            qbf = [{} for _ in range(BPC)]

            def prep_half(b, h):
                """Load x[b] half h (slab-major for early availability), cast."""
                for ci in range(CB):
                    qbf[b][ci, h] = qbf_pool.tile(
                        [128, 2048], BF16, tag="qbf", name="qbf"
                    )
                for s in range(2):
                    for ci in range(CB):
                        ns = h * 2 + s
                        xt = xf_pool.tile([128, 1024], F32, tag="xf", name="xf")
                        nc.sync.dma_start(
                            xt[:, :],
                            x[b, ci * 128 : (ci + 1) * 128,
                              ns * 1024 : (ns + 1) * 1024],
                        )
                        x_tiles[b][ci, ns] = xt
                        nc.vector.tensor_copy(
                            qbf[b][ci, h][:, s * 1024 : (s + 1) * 1024], xt[:, :]
                        )

            GK = 4  # chunks per transpose group

            def transpose_group(b, g):
                """Transpose chunks [g*GK, (g+1)*GK) into one [128, GK*512] tile.

                qt_g[p, kl*512 + ci*128 + c'] = q[ci*128+c', (g*GK+kl)*128 + p].
                Each chunk's 4 transposes land in one PSUM tile (disjoint
                column slices), drained by a single DVE copy."""
                qt_g = qts_pool.tile([128, GK * 512], BF16, tag="qts", name="qts")
                for kl in range(GK):
                    k = g * GK + kl
                    h, kk = divmod(k, NK // 2)
                    pst = ps2_pool.tile([128, 512], BF16, tag="ps2", name="qtp")
                    for ci in range(CB):
                        nc.tensor.transpose(
                            pst[:, ci * 128 : (ci + 1) * 128],
                            qbf[b][ci, h][:, kk * 128 : (kk + 1) * 128],
                            ident[:, :],
                        )
                    nc.vector.tensor_copy(
                        qt_g[:, kl * 512 : (kl + 1) * 512], pst[:, :]
                    )
                return qt_g

            def mm1_softmax(b):
                """energy -> softmax stats -> attention^T tiles (PE transpose)."""
                ps1 = [
                    ps1_pool.tile([128, 512], F32, tag="ps1", name="ps1")
                    for _ in range(CB)
                ]
                # software pipeline: transpose group g+2 while matmuling group g
                NG = NK // GK
                qt_pending = [transpose_group(b, 0), transpose_group(b, 1)]
                for g in range(NG):
                    qt_cur = qt_pending.pop(0)
                    if g + 2 < NG:
                        qt_pending.append(transpose_group(b, g + 2))
                    for kl in range(GK):
                        k = g * GK + kl
                        for mi in range(CB):
                            nc.tensor.matmul(
                                ps1[mi][:, :],
                                qt_cur[:, kl * 512 + mi * 128 : kl * 512 + (mi + 1) * 128],
                                qt_cur[:, kl * 512 : (kl + 1) * 512],
                                start=(k == 0),
                                stop=(k == NK - 1),
                            )
                rgs, a_ts = [], []
                for mi in range(CB):
                    mn = stat_pool.tile([128, 1], F32, tag="mn", name="mn")
                    nc.vector.tensor_reduce(
                        mn[:, :], ps1[mi][:, :],
                        axis=mybir.AxisListType.X, op=mybir.AluOpType.min,
                    )
                    a_t = a_pool.tile([128, 512], BF16, tag="attn", name="attn")
                    s_t = stat_pool.tile([128, 1], F32, tag="s", name="s")
                    nc.scalar.activation(
                        a_t[:, :], ps1[mi][:, :],
                        mybir.ActivationFunctionType.Exp,
                        bias=mn[:, :], scale=-1.0, accum_out=s_t[:, :],
                    )
                    a_ts.append(a_t)
                    rs = stat_pool.tile([128, 1], F32, tag="rs", name="rs")
                    nc.vector.reciprocal(rs[:, :], s_t[:, :])
                    rg_t = stat_pool.tile([128, 1], F32, tag="rg", name="rg")
                    nc.vector.tensor_tensor(
                        rg_t[:, :], rs[:, :], gb[:, :], op=mybir.AluOpType.mult
                    )
                    rgs.append(rg_t)
                # attention^T via PE transpose-mode
                at = [[None] * CB for _ in range(CB)]
                for mi in range(CB):
                    for dj in range(CB):
                        pst = ps2_pool.tile([128, 128], BF16, tag="ps2", name="atp")
                        nc.tensor.transpose(
                            pst[:, :],
                            a_ts[mi][:, dj * 128 : (dj + 1) * 128],
                            ident[:, :],
                        )
                        t_sb = at_pool.tile([128, 128], BF16, tag="att", name="att")
                        if dj % 2 == 0:
                            nc.vector.tensor_copy(t_sb[:, :], pst[:, :])
                        else:
                            nc.scalar.copy(t_sb[:, :], pst[:, :])
                        at[mi][dj] = t_sb
                return rgs, at

            def mm2_round(b, nt, rgs, at, stage):
                """One nt column of mm2 + fused epilogue; store every 2 rounds."""
                hh = nt // 4
                off = (nt % 4) * 512
                for mi in range(CB):
                    if nt % 2 == 0:
                        stage[mi] = stage_pool.tile(
                            [128, 1024], F32, tag="stage", name="stage"
                        )
                    pool, ptag = ((ps1_pool, "ps1") if (nt * CB + mi) % 2 else (ps2_pool, "ps2"))
                    ps2 = pool.tile([128, 512], F32, tag=ptag, name="ps2")
                    for dj in range(CB):
                        nc.tensor.matmul(
                            ps2[:, :],
                            at[mi][dj][:, :],
                            qbf[b][dj, hh][:, off : off + 512],
                            start=(dj == 0),
                            stop=(dj == CB - 1),
                        )
                    xsl = x_tiles[b][mi, nt // 2][
                        :, (nt % 2) * 512 : (nt % 2 + 1) * 512
                    ]
                    dst = stage[mi][:, (nt % 2) * 512 : (nt % 2 + 1) * 512]
                    if (nt * CB + mi) % 2 == 0:
                        # fused DVE op: (psum * rg) + x
                        nc.vector.scalar_tensor_tensor(
                            dst, ps2[:, :], rgs[mi][:, :], xsl,
                            op0=mybir.AluOpType.mult, op1=mybir.AluOpType.add,
                        )
                    else:
                        # ACT drains PSUM (scale), DVE adds from SBUF (2x mode)
                        t_sb = stage_pool.tile([128, 512], F32, tag="tsb", name="tsb")
                        nc.scalar.activation(
                            t_sb[:, :], ps2[:, :],
                            mybir.ActivationFunctionType.Copy,
                            bias=0.0, scale=rgs[mi][:, :],
                        )
                        nc.vector.tensor_tensor(
                            dst, t_sb[:, :], xsl, op=mybir.AluOpType.add
                        )
                    if nt % 2 == 1:
                        nc.scalar.dma_start(
                            out[b, mi * 128 : (mi + 1) * 128,
                                (nt - 1) * 512 : (nt + 1) * 512],
                            stage[mi][:, :],
                        )

            # ---- main schedule ----
            prep_half(0, 0)
            prep_half(0, 1)
            for b in range(BPC):
                rgs, at = mm1_softmax(b)
                stage = {}
                for nt in range(NT):
                    mm2_round(b, nt, rgs, at, stage)
                    # interleave next batch's prep where its slots free up
                    if b + 1 < BPC:
                        if nt == 3:
                            prep_half(b + 1, 0)
                        elif nt == NT - 1:
                            prep_half(b + 1, 1)

    nc.finalize()
    return nc


def _get_program():
    global _PROGRAM
    if _PROGRAM is None:
        _PROGRAM = _build_program()
    return _PROGRAM


def _run(x, gamma, trace=False, tmpdir=None):
    """x: [B, C, H, W] fp32, gamma: [1] fp32 -> ([B, C, H, W] fp32, exec_time_ns)"""
    x = np.ascontiguousarray(np.asarray(x, dtype=np.float32)).reshape(B, C, N)
    gamma = np.ascontiguousarray(np.asarray(gamma, dtype=np.float32)).reshape(1)
    nc = _get_program()
    in_maps = [
        {"x": x[i * BPC : (i + 1) * BPC], "gamma": gamma} for i in range(N_CORES)
    ]
    res = run_bass_kernel_spmd(
        nc, in_maps, list(range(N_CORES)), trace=trace, tmpdir=tmpdir
    )
    full = np.concatenate([res.results[i]["out"] for i in range(N_CORES)], axis=0)
    return full.reshape(B, C, H, W), res.exec_time_ns


def kernel(**inputs):
    out, _ = _run(inputs["x"], inputs["gamma"])
    return out


if __name__ == "__main__":
    rng = np.random.default_rng(0)
    x = rng.standard_normal((B, C, H, W), dtype=np.float32)
    gamma = np.zeros((1,), dtype=np.float32)
    out, t = _run(x, gamma)
    print("exec_time_ns:", t)
    print("max |out - x| (gamma=0):", np.abs(out - x).max())


# revision 11
# speedup vs baseline: 1.1742x; 1.1742x over previous
"""CAM (channel attention) kernel for Trainium2, SPMD over 8 NeuronCores.

Problem: x [16, 512, 64, 64] fp32, gamma [1] fp32.
  q = x.reshape(B, C, N);  energy = q @ q^T          (C x C, contract over N=4096)
  attention = softmax(max(energy, -1, keepdims) - energy, -1)
  out = attention @ q;  result = gamma * out + x

Sharding: data-parallel over batch. 16 batches / 8 cores = 2 batches per core.
gamma replicated. Each core computes its own C x C attention per batch.

Math: energy is symmetric, and
  softmax(m[c] - energy[c, :]) = exp(mn[c] - energy[c, :]) / sum(...)
with mn[c] = min_d energy[c, d] (jax softmax's internal max-shift turns the
row-max of (m - e) into the row-min of e). All exp args <= 0 -> no overflow.

All transposes run on the TensorEngine in transpose-mode (DMA xbar-transposes
measured ~8us each here and throttle the global in-flight DMA window, which
starves the x loads). mm1 is software-pipelined per 128-wide n-chunk:
  PE:  [transpose chunk k (4x 128x128)] [matmuls chunk k-1 (4x N=512)] ...
with tiny DVE copies draining each transposed chunk from PSUM to SBUF.

Engine layout:
  SP ring : x loads (plain, back-to-back)
  ACT ring: out stores; ACT compute: exp with fused row-sum
  DVE     : fp32->bf16 casts, qT/AT psum->sbuf copies, row-min,
            epilogue (psum*rg)+x as one fused scalar_tensor_tensor
  PE      : qT transposes, mm1, AT transposes, mm2
"""

import sys

if "/opt/trn_rl_repo" not in sys.path:
    sys.path.insert(0, "/opt/trn_rl_repo")

import numpy as np

import concourse.bacc as bacc
import concourse.mybir as mybir
import concourse.tile as tile
from concourse.bass_utils import run_bass_kernel_spmd
from concourse.masks import make_identity

# Problem constants (hardcoded; kernel.py must be self-contained).
B, C, H, W = 16, 512, 64, 64
N = H * W                      # 4096
N_CORES = 8
BPC = B // N_CORES             # batches per core = 2
CB = C // 128                  # c-blocks = 4
NK = N // 128                  # contraction chunks for mm1 = 32
NT = N // 512                  # mm2 output tiles per c-block = 8

F32 = mybir.dt.float32
BF16 = mybir.dt.bfloat16

_PROGRAM = None


def _build_program():
    nc = bacc.Bacc("TRN2", target_bir_lowering=False, debug=True)
    x = nc.declare_dram_parameter("x", [BPC, C, N], F32, isOutput=False)
    gamma = nc.declare_dram_parameter("gamma", [1], F32, isOutput=False)
    out = nc.declare_dram_parameter("out", [BPC, C, N], F32, isOutput=True)

    with tile.TileContext(nc) as tc:
        with (
            tc.tile_pool(name="xf", bufs=20) as xf_pool,
            tc.tile_pool(name="qbf", bufs=12) as qbf_pool,
            tc.tile_pool(name="qts", bufs=3) as qts_pool,
            tc.tile_pool(name="attn", bufs=4) as a_pool,
            tc.tile_pool(name="att", bufs=20) as at_pool,
            tc.tile_pool(name="stat", bufs=24) as stat_pool,
            tc.tile_pool(name="stage", bufs=6) as stage_pool,
            tc.tile_pool(name="const", bufs=1) as const_pool,
            tc.tile_pool(name="ps1", bufs=4, space="PSUM") as ps1_pool,
            tc.tile_pool(name="ps2", bufs=4, space="PSUM") as ps2_pool,
        ):
            # constants: gamma broadcast + identity for PE transposes
            gsb = const_pool.tile([1, 1], F32, tag="gsb", name="gsb")
            nc.sync.dma_start(gsb[:, :], gamma[None, :])
            gb = const_pool.tile([128, 1], F32, tag="gb", name="gb")
            nc.gpsimd.partition_broadcast(gb[:, :], gsb[:, :])
            ident = const_pool.tile([128, 128], BF16, tag="ident", name="ident")
            make_identity(nc, ident[:, :])

            # per-batch state
            x_tiles = [{} for _ in range(BPC)]
            qbf = [{} for _ in range(BPC)]

            def prep_half(b, h):
                """Load x[b] half h (slab-major for early availability), cast."""
                for ci in range(CB):
                    qbf[b][ci, h] = qbf_pool.tile(
                        [128, 2048], BF16, tag="qbf", name="qbf"
                    )
                for s in range(2):
                    for ci in range(CB):
                        ns = h * 2 + s
                        xt = xf_pool.tile([128, 1024], F32, tag="xf", name="xf")
                        nc.sync.dma_start(
                            xt[:, :],
                            x[b, ci * 128 : (ci + 1) * 128,
                              ns * 1024 : (ns + 1) * 1024],
                        )
                        x_tiles[b][ci, ns] = xt
                        nc.vector.tensor_copy(
                            qbf[b][ci, h][:, s * 1024 : (s + 1) * 1024], xt[:, :]
                        )

            GK = 4  # chunks per transpose group

            def transpose_group(b, g):
                """Transpose chunks [g*GK, (g+1)*GK) into one [128, GK*512] tile.

                qt_g[p, kl*512 + ci*128 + c'] = q[ci*128+c', (g*GK+kl)*128 + p].
                Each chunk's 4 transposes land in one PSUM tile (disjoint
                column slices), drained by a single DVE copy."""
                qt_g = qts_pool.tile([128, GK * 512], BF16, tag="qts", name="qts")
                for kl in range(GK):
                    k = g * GK + kl
                    h, kk = divmod(k, NK // 2)
                    pst = ps2_pool.tile([128, 512], BF16, tag="ps2", name="qtp")
                    for ci in range(CB):
                        nc.tensor.transpose(
                            pst[:, ci * 128 : (ci + 1) * 128],
                            qbf[b][ci, h][:, kk * 128 : (kk + 1) * 128],
                            ident[:, :],
                        )
                    nc.vector.tensor_copy(
                        qt_g[:, kl * 512 : (kl + 1) * 512], pst[:, :]
                    )
                return qt_g

            def mm1_softmax(b):
                """energy -> softmax stats -> attention^T tiles (PE transpose)."""
                ps1 = [
                    ps1_pool.tile([128, 512], F32, tag="ps1", name="ps1")
                    for _ in range(CB)
                ]
                # software pipeline: transpose group g+2 while matmuling group g
                NG = NK // GK
                qt_pending = [transpose_group(b, 0), transpose_group(b, 1)]
                for g in range(NG):
                    qt_cur = qt_pending.pop(0)
                    if g + 2 < NG:
                        qt_pending.append(transpose_group(b, g + 2))
                    for kl in range(GK):
                        k = g * GK + kl
                        for mi in range(CB):
                            nc.tensor.matmul(
                                ps1[mi][:, :],
                                qt_cur[:, kl * 512 + mi * 128 : kl * 512 + (mi + 1) * 128],
                                qt_cur[:, kl * 512 : (kl + 1) * 512],
                                start=(k == 0),
                                stop=(k == NK - 1),
                            )
                rgs, a_ts = [], []
                for mi in range(CB):
                    mn = stat_pool.tile([128, 1], F32, tag="mn", name="mn")
                    nc.vector.tensor_reduce(
                        mn[:, :], ps1[mi][:, :],
                        axis=mybir.AxisListType.X, op=mybir.AluOpType.min,
                    )
                    a_t = a_pool.tile([128, 512], BF16, tag="attn", name="attn")
                    s_t = stat_pool.tile([128, 1], F32, tag="s", name="s")
                    nc.scalar.activation(
                        a_t[:, :], ps1[mi][:, :],
                        mybir.ActivationFunctionType.Exp,
                        bias=mn[:, :], scale=-1.0, accum_out=s_t[:, :],
                    )
                    a_ts.append(a_t)
                    rs = stat_pool.tile([128, 1], F32, tag="rs", name="rs")
                    nc.vector.reciprocal(rs[:, :], s_t[:, :])
                    rg_t = stat_pool.tile([128, 1], F32, tag="rg", name="rg")
                    nc.vector.tensor_tensor(
                        rg_t[:, :], rs[:, :], gb[:, :], op=mybir.AluOpType.mult
                    )
                    rgs.append(rg_t)
                # attention^T via PE transpose-mode
                at = [[None] * CB for _ in range(CB)]
                for mi in range(CB):
                    for dj in range(CB):
                        pst = ps2_pool.tile([128, 128], BF16, tag="ps2", name="atp")
                        nc.tensor.transpose(
                            pst[:, :],
                            a_ts[mi][:, dj * 128 : (dj + 1) * 128],
                            ident[:, :],
                        )
                        t_sb = at_pool.tile([128, 128], BF16, tag="att", name="att")
                        if dj % 2 == 0:
                            nc.vector.tensor_copy(t_sb[:, :], pst[:, :])
                        else:
                            nc.scalar.copy(t_sb[:, :], pst[:, :])
                        at[mi][dj] = t_sb
                return rgs, at

            def mm2_round(b, nt, rgs, at, stage):
                """One nt column of mm2 + fused epilogue; store every 2 rounds."""
                hh = nt // 4
                off = (nt % 4) * 512
                for mi in range(CB):
                    if nt % 2 == 0:
                        stage[mi] = stage_pool.tile(
                            [128, 1024], F32, tag="stage", name="stage"
                        )
                    pool, ptag = ((ps1_pool, "ps1") if (nt * CB + mi) % 2 else (ps2_pool, "ps2"))
                    ps2 = pool.tile([128, 512], F32, tag=ptag, name="ps2")
                    for dj in range(CB):
                        nc.tensor.matmul(
                            ps2[:, :],
                            at[mi][dj][:, :],
                            qbf[b][dj, hh][:, off : off + 512],
                            start=(dj == 0),
                            stop=(dj == CB - 1),
                        )
                    xsl = x_tiles[b][mi, nt // 2][
                        :, (nt % 2) * 512 : (nt % 2 + 1) * 512
                    ]
                    dst = stage[mi][:, (nt % 2) * 512 : (nt % 2 + 1) * 512]
                    nc.vector.scalar_tensor_tensor(
                        dst, ps2[:, :], rgs[mi][:, :], xsl,
                        op0=mybir.AluOpType.mult, op1=mybir.AluOpType.add,
                    )
                    if nt % 2 == 1:
                        nc.scalar.dma_start(
                            out[b, mi * 128 : (mi + 1) * 128,
                                (nt - 1) * 512 : (nt + 1) * 512],
                            stage[mi][:, :],
                        )

            # ---- main schedule ----
            prep_half(0, 0)
            prep_half(0, 1)
            for b in range(BPC):
                rgs, at = mm1_softmax(b)
                stage = {}
                for nt in range(NT):
                    mm2_round(b, nt, rgs, at, stage)
                    # interleave next batch's prep where its slots free up
                    if b + 1 < BPC:
                        if nt == 3:
                            prep_half(b + 1, 0)
                        elif nt == NT - 1:
                            prep_half(b + 1, 1)

    nc.finalize()
    return nc


def _get_program():
    global _PROGRAM
    if _PROGRAM is None:
        _PROGRAM = _build_program()
    return _PROGRAM


def _run(x, gamma, trace=False, tmpdir=None):
    """x: [B, C, H, W] fp32, gamma: [1] fp32 -> ([B, C, H, W] fp32, exec_time_ns)"""
    x = np.ascontiguousarray(np.asarray(x, dtype=np.float32)).reshape(B, C, N)
    gamma = np.ascontiguousarray(np.asarray(gamma, dtype=np.float32)).reshape(1)
    nc = _get_program()
    in_maps = [
        {"x": x[i * BPC : (i + 1) * BPC], "gamma": gamma} for i in range(N_CORES)
    ]
    res = run_bass_kernel_spmd(
        nc, in_maps, list(range(N_CORES)), trace=trace, tmpdir=tmpdir
    )
    full = np.concatenate([res.results[i]["out"] for i in range(N_CORES)], axis=0)
    return full.reshape(B, C, H, W), res.exec_time_ns


def kernel(**inputs):
    out, _ = _run(inputs["x"], inputs["gamma"])
    return out


if __name__ == "__main__":
    rng = np.random.default_rng(0)
    x = rng.standard_normal((B, C, H, W), dtype=np.float32)
    gamma = np.zeros((1,), dtype=np.float32)
    out, t = _run(x, gamma)
    print("exec_time_ns:", t)
    print("max |out - x| (gamma=0):", np.abs(out - x).max())
